# revision 1
# baseline (speedup 1.0000x reference)
"""Trainium2 Bass kernel for nn_Diagnet (S=1024, B=64, I=512, H=2048, O=512).

    u = einsum('sbi,hi->sbh', X, W_ih)
    h_{t} = |u_t + hh * h_{t-1}|   (scan over S, only final h needed)
    Y = h_final @ W_ho.T + b_ho

Strategy (8 NeuronCores, data-parallel over batch, 8 batch rows per core):

* H lanes are permuted so hh is sorted descending and split into 16
  chunks of 128.  The recurrence is a contraction with per-lane factor
  a=hh<1, so a chunk whose largest a satisfies a^K < 1e-10 only needs
  the last K steps: the input->hidden GEMM and the scan skip everything
  earlier (this is exact to ~1e-10 relative, far below fp32 noise).
* Within each 64-step block the state is kept pre-scaled as
  m_tau = a^(63-tau) * h.  Then the step is a multiply-free
  m = |m + a^(63-tau) u_t|, applied by a custom fused DVE op
  (out = |in0 + in1|), one instruction per step over all active chunks.
  Entering a block multiplies the state once by a^64.  Scales a^(63-tau)
  fold into the PSUM->SBUF move of the GEMM output (one tensor_tensor
  multiply).  Underflow of a^64 for small-a lanes reproduces the
  truncation automatically, and no overflow is possible (scales <= 1).
* GEMM: X is pre-tiled host-side into [block, i-chunk, 128, (b,t)]
  (contraction dim on partitions), multiplied against host-transposed
  W_ih^T in fp32.  PSUM layout [h, (b,t)] hands each scan step a
  contiguous slice after a fused scale+move to SBUF.
* Final projection: h_final tiles (already [h,b] on chip) are the
  stationary operand against host-transposed W_ho^T; bias added on DVE.
"""

import math
import os

from contextlib import ExitStack

import numpy as np

S, B, I, H, O = 1024, 64, 512, 2048, 512
NCORES = 8
BC = B // NCORES  # 8 batch rows per core
TB = 64  # time block == scan window
NBLK = S // TB  # 16
NCH = H // 128  # 16 h-chunks
LN_TRUNC = 23.03  # a^K <= e^-23 ~ 1e-10 -> truncate

_CACHE = {}


def _register_abs_add():
    import concourse.dve_ops as dve_ops
    from concourse.dve_spec import Spec, Src0, Src1, Zero, maxx, lower
    from concourse.dve_uop import DveOpSpec

    for op in dve_ops.OPS:
        if op.name == "ABS_ADD_ANT":
            return op
    x = Src0 + Src1
    spec = Spec(
        body=maxx(x, Zero - x),
        reference=lambda in0, in1, s0, s1, imm2: np.abs(
            in0.astype(np.float32) + in1.astype(np.float32)
        ),
    )
    row = max(dve_ops._SUB_OPCODE_FOR_NAME.values()) + 1
    assert row < 0x20
    shas = {}
    for ver in ("v3", "v4"):
        s = DveOpSpec(name="ABS_ADD_ANT", opcode=row, uops=lower(spec, ver=ver), rd1_en=True)
        shas[ver] = s.sha(ver)
    op = dve_ops.DveOp("ABS_ADD_ANT", spec, subdim=False, uops_sha=shas)
    dve_ops._SUB_OPCODE_FOR_NAME["ABS_ADD_ANT"] = row
    dve_ops.OPS.append(op)
    dve_ops.CUSTOM_DVE_SPECS["ABS_ADD_ANT"] = spec
    return op


def _make_plan(hh):
    a = np.maximum(np.abs(hh.astype(np.float64)), 1e-30)
    # jax uniform is [0,1); abs is a no-op safeguard.
    perm = np.argsort(-a, kind="stable")
    a_s = a[perm]
    first_block = []
    for g in range(NCH):
        amax = a_s[g * 128]
        if amax >= math.exp(-LN_TRUNC / S):
            kg = S
        else:
            kg = min(S, int(math.ceil(LN_TRUNC / math.log(1.0 / amax))))
        kg = min(S, ((kg + TB - 1) // TB) * TB)
        first_block.append(NBLK - kg // TB)
    # chunks sorted by a desc -> first_block nondecreasing -> active set is
    # always a chunk prefix.
    assert all(
        first_block[g] <= first_block[g + 1] for g in range(NCH - 1)
    ), first_block
    ag = a_s.reshape(NCH, 128).T  # [128, NCH] lane a per chunk
    tau = np.arange(TB)
    sc = ag[:, :, None] ** (TB - 1 - tau)[None, None, :]  # [128, NCH, TB]
    a64 = np.repeat(ag**TB, BC, axis=1)  # [128, NCH*BC]
    return {
        "perm": perm,
        "first_block": tuple(first_block),
        "SC": sc.reshape(128, NCH * TB).astype(np.float32),
        "A64": a64.astype(np.float32),
    }


def _build(first_block, use_f32r):
    import concourse.mybir as mybir
    import concourse.tile as tile
    from concourse import bacc
    from concourse.bass import ds

    ABS_ADD = _register_abs_add()
    f32 = mybir.dt.float32
    gemm_dt = mybir.dt.float32r if use_f32r else f32

    nc = bacc.Bacc("TRN2", target_bir_lowering=False, debug=False, num_devices=NCORES)
    X = nc.dram_tensor("X", [NBLK, I // 128, 128, TB * BC], gemm_dt, kind="ExternalInput").ap()
    WIHT = nc.dram_tensor("WIHT", [I, H], gemm_dt, kind="ExternalInput").ap()
    WHOT = nc.dram_tensor("WHOT", [H, O], f32, kind="ExternalInput").ap()
    BIAS = nc.dram_tensor("BIAS", [BC, O], f32, kind="ExternalInput").ap()
    SC = nc.dram_tensor("SC", [128, NCH * TB], f32, kind="ExternalInput").ap()
    A64 = nc.dram_tensor("A64", [128, NCH * BC], f32, kind="ExternalInput").ap()
    Y = nc.dram_tensor("Y", [BC, O], f32, kind="ExternalOutput").ap()

    NI = I // 128  # 4 i-chunks

    with tile.TileContext(nc) as tc:
        with ExitStack() as ctx:
            consts = ctx.enter_context(tc.tile_pool(name="consts", bufs=1))
            xtpool = ctx.enter_context(tc.tile_pool(name="xt", bufs=3))
            upool = ctx.enter_context(tc.tile_pool(name="ubuf", bufs=1))
            ypool = ctx.enter_context(tc.tile_pool(name="yout", bufs=1))
            gpool = ctx.enter_context(tc.tile_pool(name="gpsum", bufs=int(os.environ.get("DIAG_GP", "4")), space="PSUM"))
            fpool = ctx.enter_context(tc.tile_pool(name="fpsum", bufs=1, space="PSUM"))

            # constants
            wiht = [consts.tile([128, H], gemm_dt, tag=f"wiht{ic}", name=f"wiht{ic}") for ic in range(NI)]
            for ic in range(NI):
                nc.sync.dma_start(wiht[ic][:], WIHT[ds(ic * 128, 128), :])
            sc_t = consts.tile([128, NCH * TB], f32, tag="sc", name="sc_t")
            nc.sync.dma_start(sc_t[:], SC)
            a64_t = consts.tile([128, NCH * BC], f32, tag="a64", name="a64_t")
            nc.sync.dma_start(a64_t[:], A64)
            m_t = consts.tile([128, NCH * BC], f32, tag="state", name="m_t")
            nc.vector.memset(m_t[:], 0.0)

            acts = [sum(1 for fb in first_block if fb <= kb) for kb in range(NBLK)]
            assert all(a >= 1 for a in acts)
            u_tiles = [None] * NBLK

            def produce(kb):
                act = acts[kb]
                # --- load pre-transposed X tiles [i, (b,t)] ---
                xt = []
                for ic in range(NI):
                    xt_ic = xtpool.tile([128, TB * BC], gemm_dt, tag=f"xt{ic}", name=f"xt_{kb}_{ic}")
                    nc.sync.dma_start(xt_ic[:], X[kb, ic])
                    xt.append(xt_ic)
                # u buffer for this block: [128, (tau, active-chunk, b)]
                u_t = upool.tile([128, TB * act * BC], f32, tag=f"u{kb}", name=f"u_{kb}")
                u_tiles[kb] = u_t
                for g in range(act):
                    ps = gpool.tile([128, TB * BC], f32, tag="gp", name=f"gp_{kb}_{g}")
                    for ic in range(NI):
                        nc.tensor.matmul(
                            ps[:],
                            wiht[ic][:, ds(g * 128, 128)],
                            xt[ic][:],
                            start=(ic == 0),
                            stop=(ic == NI - 1),
                        )
                    # scaled move psum->sbuf:
                    # u_t[p, tau*act*BC + g*BC + b] = ps[p, b*TB+tau]*SC[p,g*TB+tau]
                    dst = u_t[:].rearrange("p (t c) -> p t c", t=TB)[
                        :, :, ds(g * BC, BC)
                    ]
                    srcp = ps[:].rearrange("p (b t) -> p t b", b=BC)
                    scl = sc_t[:, ds(g * TB, TB)].broadcast_to([128, TB, BC])
                    nc.vector.tensor_tensor(dst, srcp, scl, mybir.AluOpType.mult)

            def scan(kb):
                act = acts[kb]
                na = act * BC
                u_t = u_tiles[kb]
                nc.gpsimd.tensor_tensor(
                    m_t[:, 0:na], m_t[:, 0:na], a64_t[:, 0:na], mybir.AluOpType.mult
                )
                for tau in range(TB):
                    nc.vector._custom_dve(
                        ABS_ADD,
                        out=m_t[:, 0:na],
                        in0=m_t[:, 0:na],
                        in1=u_t[:, ds(tau * act * BC, na)],
                    )

            LAG = int(os.environ.get("DIAG_LAG", "2"))
            if os.environ.get("DIAG_ORDER", "seq") == "front":
                # front-load the heaviest (latest) blocks' GEMMs to keep the
                # PE dense/warm while the serial scan chain progresses.
                heavy = [NBLK - 1, NBLK - 2]
                order = heavy + [kb for kb in range(NBLK) if kb not in heavy]
            else:
                order = list(range(NBLK))
            scanned = 0
            produced = set()

            def scan_ready_upto(limit):
                nonlocal scanned
                while scanned < limit and scanned in produced:
                    scan(scanned)
                    scanned += 1

            for i, kb in enumerate(order):
                produce(kb)
                produced.add(kb)
                scan_ready_upto(i + 1 - LAG)
            scan_ready_upto(NBLK)
            assert scanned == NBLK

            # --- final projection: Y = h^T @ WHOT + bias ---
            whot = [consts.tile([128, O], f32, tag=f"whot{g}", name=f"whot{g}") for g in range(NCH)]
            for g in range(NCH):
                nc.sync.dma_start(whot[g][:], WHOT[ds(g * 128, 128), :])
            bias_t = ypool.tile([BC, O], f32, tag="bias", name="bias_t")
            nc.sync.dma_start(bias_t[:], BIAS)
            psy = fpool.tile([BC, O], f32, tag="fy", name="psy")
            for g in range(NCH):
                nc.tensor.matmul(
                    psy[:],
                    m_t[:, ds(g * BC, BC)],
                    whot[g][:],
                    start=(g == 0),
                    stop=(g == NCH - 1),
                )
            y_t = ypool.tile([BC, O], f32, tag="y", name="y_t")
            nc.vector.tensor_tensor(y_t[:], psy[:], bias_t[:], mybir.AluOpType.add)
            nc.sync.dma_start(Y, y_t[:])
    nc.compile()
    return nc


def _get_program(first_block, use_f32r):
    key = (first_block, use_f32r, os.environ.get("DIAG_LAG"), os.environ.get("DIAG_GP"), os.environ.get("DIAG_ORDER"))
    if key not in _CACHE:
        _CACHE[key] = _build(first_block, use_f32r)
    return _CACHE[key]


def _round_f32r(x):
    """Round fp32 array to fp32r (s8e11) representable values."""
    u = np.ascontiguousarray(x).view(np.uint32)
    r = ((u.astype(np.uint64) + 0x800) & 0xFFFFF000).astype(np.uint32)
    return r.view(np.float32).reshape(x.shape)


def _ensure_ntff_hook():
    """Provide antenv.axon_hooks (absent in this image) so trace=True works."""
    import sys
    import types

    if "antenv.axon_hooks" in sys.modules:
        return True
    try:
        import antenv

        mod = types.ModuleType("antenv.axon_hooks")
        mod._hook = None

        def set_axon_ntff_profile_hook(h):
            mod._hook = h

        def get_axon_ntff_profile_hook():
            return mod._hook

        mod.set_axon_ntff_profile_hook = set_axon_ntff_profile_hook
        mod.get_axon_ntff_profile_hook = get_axon_ntff_profile_hook
        sys.modules["antenv.axon_hooks"] = mod
        antenv.axon_hooks = mod

        from trn_agent_boot.trn_boot import _ntff_profile_via_ctypes

        hook = _ntff_profile_via_ctypes("/opt/axon/libaxon_pjrt.so")
        mod.set_axon_ntff_profile_hook(hook)
        return hook is not None
    except Exception:
        return False


def kernel(X, W_ih, hh, W_ho, b_ho):
    from concourse import bass_utils

    X = np.asarray(X, dtype=np.float32)
    W_ih = np.asarray(W_ih, dtype=np.float32)
    hh = np.asarray(hh, dtype=np.float32)
    W_ho = np.asarray(W_ho, dtype=np.float32)
    b_ho = np.asarray(b_ho, dtype=np.float32)

    use_f32r = bool(int(os.environ.get("DIAG_F32R", "0")))
    plan = _make_plan(hh)
    perm = plan["perm"]
    nc = _get_program(plan["first_block"], use_f32r)

    wiht = np.ascontiguousarray(W_ih[perm].T)  # [I, H]
    if use_f32r:
        wiht = _round_f32r(wiht)
    whot = np.ascontiguousarray(W_ho[:, perm].T)  # [H, O]
    bias = np.tile(b_ho[None, :], (BC, 1)).astype(np.float32)

    common = {
        "WIHT": wiht,
        "WHOT": whot,
        "BIAS": bias,
        "SC": plan["SC"],
        "A64": plan["A64"],
    }
    in_maps = []
    for m in range(NCORES):
        im = dict(common)
        xm = X[:, m * BC : (m + 1) * BC, :]  # [S, BC, I]
        # device tile layout [NBLK, NI, 128(i), (b, tau)]
        xt = xm.transpose(2, 1, 0).reshape(I // 128, 128, BC, NBLK, TB)
        xt = np.ascontiguousarray(xt.transpose(3, 0, 1, 2, 4)).reshape(
            NBLK, I // 128, 128, TB * BC
        )
        if use_f32r:
            xt = _round_f32r(xt)
        im["X"] = xt
        in_maps.append(im)

    trace = bool(int(os.environ.get("DIAG_TRACE", "0")))
    if trace:
        trace = _ensure_ntff_hook()
    res = None
    for attempt in range(3):
        try:
            res = bass_utils.run_bass_kernel_spmd(
                nc,
                in_maps,
                core_ids=list(range(NCORES)),
                trace=trace,
                tmpdir=os.environ.get("DIAG_TRACE_DIR") or None,
            )
            break
        except Exception:
            if attempt == 2:
                raise
            trace = False  # retry without profiling
    if res.exec_time_ns is not None:
        kernel.last_exec_time_ns = res.exec_time_ns
        kernel.last_mean_exec_time_ns = res.mean_exec_time_ns
    Yfull = np.concatenate([r["Y"] for r in res.results], axis=0)
    return Yfull


kernel.last_exec_time_ns = None
kernel.last_mean_exec_time_ns = None



# revision 5
# speedup vs baseline: 3.3813x; 3.3813x over previous
"""Trainium2 Bass kernel for nn_Diagnet (S=1024, B=64, I=512, H=2048, O=512).

    u = einsum('sbi,hi->sbh', X, W_ih)
    h_t = |u_t + hh * h_{t-1}|   (scan over S, only final h needed)
    Y = h_final @ W_ho.T + b_ho

Strategy (8 NeuronCores, data-parallel over batch, 8 batch rows/core):

* H lanes permuted so hh is sorted descending, split into 16 chunks of
  128.  Chunk g only needs the last kg steps where amax(g)^kg ~ 1e-5
  (truncation, exact far below fp16 noise).  kg rounds up to 64-step
  blocks; chunks with kg == 64 are "shorts" (window = final block only).
* GEMM in fp16 (X, W_ih host-cast).  PSUM fp32, then the Activation
  engine copies each [128, (b,tau)] tile into a per-column fp16 u
  buffer (the only engine-level data movement).
* The scan runs on the DVE as a custom instruction ABS_SCAN_ANT:
      state_k = |state_{k-1} - u_k * scn_k|   (ABSOLUTE_DIFF prefix scan)
  with scn = NEGATED pre-scales -a^(K-1-t), so state_k tracks the
  pre-scaled recurrence m_t = a^(K-1-t) h_t and the final element IS
  h_final.  A mask (Idx >= K-1) + ADD-accum extracts the final state
  into m[:, (g,b)], which is also the s0 carry for the next piece of
  the same column.  One instruction covers up to a whole window.
* Shorts all merge into ONE scan stream per batch column: segments of
  [24 flush elements (scales -128*2^-j fold any state to <1e-5) +
  64-step window].  Uniform 88-element segments put every chunk's
  final state at stride 88; one strided DVE copy gathers them into m.
* Block production order: 15 FIRST (it ends every window -> shorts and
  all final pieces unblock early), then the mid chunks' blocks
  (13, 14), then 0..12 for chunk 0's piece-chasing.  The tail after
  the last GEMM is just chunk 0's final 64-step piece + output.
* Final projection: per chunk, m -> fp16, matmul vs fp16 W_ho^T
  accumulated in one PSUM bank as chunks complete; bias on DVE.
"""

import math
import os

from contextlib import ExitStack

import numpy as np

S, B, I, H, O = 1024, 64, 512, 2048, 512
NCORES = 8
BC = B // NCORES  # 8 batch rows per core
TB = 64  # time block
NBLK = S // TB  # 16
NCH = H // 128  # 16 h-chunks
NI = I // 128  # 4 i-chunks
LN_TRUNC = 11.5  # a^K <= e^-11.5 ~ 1e-5 -> truncate (gate is 2e-2)
NFLUSH = 24  # 128*2^-24 ~ 7.6e-6 residual after flush
SEG = NFLUSH + TB  # 88-element short segment
PIECE_BLKS = 5  # max blocks per long-chunk scan piece (before final)

_CACHE = {}


def _register_abs_scan():
    import concourse.dve_ops as dve_ops
    from concourse.dve_spec import Spec, Src0, Src1, Zero, C0, C1, scan, Idx, lower, AluOp
    from concourse.dve_uop import DveOpSpec

    for op in dve_ops.OPS:
        if op.name == "ABS_SCAN_ANT":
            return op

    def ref(in0, in1, s0, s1, imm2):
        x = in0.astype(np.float32) * in1.astype(np.float32)
        st = np.broadcast_to(np.asarray(s0, np.float32), x[:, 0].shape).copy()
        out = np.empty_like(x, dtype=np.float32)
        for k in range(x.shape[-1]):
            st = np.abs(st - x[:, k])
            out[:, k] = st * (k >= s1)
        return out

    state = scan(AluOp.ABSOLUTE_DIFF, Src0 * Src1, init=C0)
    spec = Spec(body=state * (Idx >= C1), accum=AluOp.ADD, accum_init=Zero, reference=ref)
    row = max(dve_ops._SUB_OPCODE_FOR_NAME.values()) + 1
    assert row < 0x20
    shas = {}
    for ver in ("v3", "v4"):
        s = DveOpSpec(name="ABS_SCAN_ANT", opcode=row, uops=lower(spec, ver=ver), rd1_en=True)
        shas[ver] = s.sha(ver)
    op = dve_ops.DveOp("ABS_SCAN_ANT", spec, subdim=False, uops_sha=shas)
    dve_ops._SUB_OPCODE_FOR_NAME["ABS_SCAN_ANT"] = row
    dve_ops.OPS.append(op)
    dve_ops.CUSTOM_DVE_SPECS["ABS_SCAN_ANT"] = spec
    return op


def _make_plan(hh):
    a = np.maximum(np.abs(hh.astype(np.float64)), 1e-30)
    perm = np.argsort(-a, kind="stable")
    a_s = a[perm]
    kgs = []
    for g in range(NCH):
        amax = a_s[g * 128]
        if amax >= math.exp(-LN_TRUNC / S):
            kg = S
        else:
            kg = min(S, int(math.ceil(LN_TRUNC / math.log(1.0 / amax))))
        kg = max(TB, min(S, ((kg + TB - 1) // TB) * TB))
        kgs.append(kg)
    assert all(kgs[g] >= kgs[g + 1] for g in range(NCH - 1)), kgs
    ag = a_s.reshape(NCH, 128)  # [chunk, lane]

    longs = [g for g in range(NCH) if kgs[g] > TB]
    shorts = [g for g in range(NCH) if kgs[g] == TB]
    NSH = len(shorts)

    # SCN: negated pre-scales. longs first (kg cols each), then the merged
    # shorts stream (NSH segments of [NFLUSH flush + TB window]).
    scn_off = {}
    off = 0
    for g in longs:
        scn_off[g] = off
        off += kgs[g]
    scn_shorts_off = off
    off += NSH * SEG
    scn = np.zeros((128, off), dtype=np.float64)
    for g in longs:
        kg = kgs[g]
        t = np.arange(kg)
        scn[:, scn_off[g] : scn_off[g] + kg] = -(ag[g][:, None] ** (kg - 1 - t)[None, :])
    # flush elements fold |state - 128*2^-j| -> state collapses to <1e-5;
    # POSITIVE sign (the window scales are negated, these must not be).
    flush = 128.0 * (0.5 ** np.arange(NFLUSH))
    for i, g in enumerate(shorts):
        base = scn_shorts_off + i * SEG
        scn[:, base : base + NFLUSH] = flush[None, :]
        t = np.arange(TB)
        scn[:, base + NFLUSH : base + SEG] = -(ag[g][:, None] ** (TB - 1 - t)[None, :])
    scn = scn.astype(np.float32)

    # u layout: per long chunk g: 8 columns of kg; then shorts: 8 columns
    # of NSH*SEG.
    u_off = {}
    off = 0
    for g in longs:
        u_off[g] = off
        off += BC * kgs[g]
    u_shorts_off = off
    off += BC * NSH * SEG
    u_cols = off

    fb = {g: NBLK - kgs[g] // TB for g in longs}
    # block production order: 15 first, then mid-chunk blocks ascending,
    # then chunk-0-only blocks ascending.
    mids = longs[1:]
    mid_lo = min((fb[g] for g in mids), default=NBLK - 1)
    order = [NBLK - 1]
    order += [kb for kb in range(mid_lo, NBLK - 1)]
    order += [kb for kb in range(fb[longs[0]], mid_lo)]
    assert sorted(order) == list(range(fb[longs[0]], NBLK)), (order, fb)

    # scan pieces per long chunk: groups of <= PIECE_BLKS blocks over
    # [fb, 14], then block 15 alone (the tail piece).
    pieces = {}
    for g in longs:
        blks = list(range(fb[g], NBLK - 1))
        grps = [blks[i : i + PIECE_BLKS] for i in range(0, len(blks), PIECE_BLKS)]
        grps.append([NBLK - 1])
        pieces[g] = grps

    return {
        "perm": perm,
        "kgs": tuple(kgs),
        "longs": tuple(longs),
        "shorts": tuple(shorts),
        "scn_off": scn_off,
        "scn_shorts_off": scn_shorts_off,
        "u_off": u_off,
        "u_shorts_off": u_shorts_off,
        "u_cols": u_cols,
        "fb": fb,
        "order": tuple(order),
        "pieces": pieces,
        "SCN": scn,
    }


def _build(plan_key, plan):
    import concourse.mybir as mybir
    import concourse.tile as tile
    from concourse import bacc
    from concourse.bass import ds

    ABS_SCAN = _register_abs_scan()
    f32 = mybir.dt.float32
    f16 = mybir.dt.float16

    kgs = plan["kgs"]
    longs = plan["longs"]
    shorts = plan["shorts"]
    NSH = len(shorts)
    scn_off = plan["scn_off"]
    scn_shorts_off = plan["scn_shorts_off"]
    u_off = plan["u_off"]
    u_shorts_off = plan["u_shorts_off"]
    u_cols = plan["u_cols"]
    fb = plan["fb"]
    order = plan["order"]
    pieces = plan["pieces"]
    scn_cols = plan["SCN"].shape[1]
    fb0 = fb[longs[0]]

    nc = bacc.Bacc("TRN2", target_bir_lowering=False, debug=False, num_devices=NCORES)
    X = nc.dram_tensor("X", [NBLK, NI, 128, TB * BC], f16, kind="ExternalInput").ap()
    WIHT = nc.dram_tensor("WIHT", [I, H], f16, kind="ExternalInput").ap()
    WHOT = nc.dram_tensor("WHOT", [H, O], f16, kind="ExternalInput").ap()
    BIAS = nc.dram_tensor("BIAS", [BC, O], f32, kind="ExternalInput").ap()
    SCN = nc.dram_tensor("SCN", [128, scn_cols], f32, kind="ExternalInput").ap()
    Y = nc.dram_tensor("Y", [BC, O], f32, kind="ExternalOutput").ap()

    with tile.TileContext(nc) as tc:
        with ExitStack() as ctx:
            consts = ctx.enter_context(tc.tile_pool(name="consts", bufs=1))
            gpool = ctx.enter_context(tc.tile_pool(name="gpsum", bufs=7, space="PSUM"))
            fpool = ctx.enter_context(tc.tile_pool(name="fpsum", bufs=1, space="PSUM"))

            wiht = [consts.tile([128, H], f16, tag=f"wiht{ic}", name=f"wiht{ic}") for ic in range(NI)]
            for ic in range(NI):
                nc.sync.dma_start(wiht[ic][:], WIHT[ds(ic * 128, 128), :])
            scn_t = consts.tile([128, scn_cols], f32, tag="scn", name="scn_t")
            nc.sync.dma_start(scn_t[:], SCN)
            xt = {}
            for kb in order:
                for ic in range(NI):
                    t = consts.tile([128, TB * BC], f16, tag=f"xt{kb}_{ic}", name=f"xt{kb}_{ic}")
                    nc.sync.dma_start(t[:], X[kb, ic])
                    xt[(kb, ic)] = t
            whot = [consts.tile([128, O], f16, tag=f"whot{g}", name=f"whot{g}") for g in range(NCH)]
            for g in range(NCH):
                nc.sync.dma_start(whot[g][:], WHOT[ds(g * 128, 128), :])
            bias_t = consts.tile([BC, O], f32, tag="bias", name="bias_t")
            nc.sync.dma_start(bias_t[:], BIAS)

            u_t = consts.tile([128, u_cols], f16, tag="u", name="u_t")
            m_t = consts.tile([128, NCH * BC], f32, tag="m", name="m_t")
            mh_t = consts.tile([128, NCH * BC], f16, tag="mh", name="mh_t")
            scr = consts.tile([128, max(NSH * SEG, PIECE_BLKS * TB)], f32, tag="scr", name="scr")
            nc.vector.memset(m_t[:], 0.0)
            # flush cells of the shorts u region read as 1.0 (scales carry
            # the flush values); windows get overwritten by copies.
            if NSH:
                nc.gpsimd.memset(u_t[:, ds(u_shorts_off, BC * NSH * SEG)], 1.0)

            psy = fpool.tile([BC, O], f32, tag="fy", name="psy")

            def u3d(base_off, total, t):
                return u_t[:, ds(base_off, total)].rearrange(
                    "p (b t) -> p b t", b=BC, t=t
                )

            def produce(kb):
                # GEMM + Act-copy for every chunk active at block kb
                active = [g for g in longs if fb[g] <= kb]
                if kb == NBLK - 1:
                    active = active + list(shorts)
                for g in active:
                    ps = gpool.tile([128, TB * BC], f32, tag="gp", name=f"gp_{kb}_{g}")
                    for ic in range(NI):
                        nc.tensor.matmul(
                            ps[:],
                            wiht[ic][:, ds(g * 128, 128)],
                            xt[(kb, ic)][:],
                            start=(ic == 0),
                            stop=(ic == NI - 1),
                        )
                    src = ps[:].rearrange("p (b t) -> p b t", b=BC)
                    if g in shorts:
                        i = shorts.index(g)
                        dst = u3d(u_shorts_off, BC * NSH * SEG, NSH * SEG)[
                            :, :, ds(i * SEG + NFLUSH, TB)
                        ]
                    else:
                        kg = kgs[g]
                        dst = u3d(u_off[g], BC * kg, kg)[
                            :, :, ds((kb - fb[g]) * TB, TB)
                        ]
                    nc.scalar.copy(dst, src)

            def scan_piece(g, blks):
                kg = kgs[g]
                lo = (blks[0] - fb[g]) * TB
                n = len(blks) * TB
                for b in range(BC):
                    nc.vector._custom_dve(
                        ABS_SCAN,
                        out=scr[:, ds(0, n)],
                        in0=u_t[:, ds(u_off[g] + b * kg + lo, n)],
                        in1=scn_t[:, ds(scn_off[g] + lo, n)],
                        s0=m_t[:, ds(g * BC + b, 1)],
                        s1=float(n - 1),
                        accum_out=m_t[:, ds(g * BC + b, 1)],
                    )

            def scan_shorts():
                n = NSH * SEG
                for b in range(BC):
                    nc.vector._custom_dve(
                        ABS_SCAN,
                        out=scr[:, ds(0, n)],
                        in0=u_t[:, ds(u_shorts_off + b * n, n)],
                        in1=scn_t[:, ds(scn_shorts_off, n)],
                        s0=m_t[:, ds(shorts[0] * BC + b, 1)],
                        s1=float(-1.0),  # no mask; finals gathered from scr
                    )
                    # gather finals: scr[:, i*SEG + SEG-1] -> m[:, g_i*BC+b]
                    src = scr[:, ds(0, n)].rearrange("p (s o) -> p s o", s=NSH, o=SEG)[
                        :, :, ds(SEG - 1, 1)
                    ]
                    dst = m_t[:, ds(shorts[0] * BC, NSH * BC)].rearrange(
                        "p (s o) -> p s o", s=NSH, o=BC
                    )[:, :, ds(b, 1)]
                    nc.vector.tensor_scalar_mul(dst, src, 1.0)

            # ---- schedule ----
            # Scans are issued as their blocks complete; final projection
            # matmuls are ALL deferred to the end (PE executes in order — an
            # early final-MM would stall the PE queue on the DVE pipeline).
            produced = []
            scanned_pieces = {g: 0 for g in longs}
            shorts_done = [False]

            def try_scans():
                if not shorts_done[0] and NBLK - 1 in produced and NSH:
                    scan_shorts()
                    shorts_done[0] = True
                for g in longs:
                    grps = pieces[g]
                    while scanned_pieces[g] < len(grps):
                        blks = grps[scanned_pieces[g]]
                        if not all(kb in produced for kb in blks):
                            break
                        # defer the final (block-15) piece of the FIRST long
                        # chunk until after all GEMMs (it is the tail).
                        if (
                            g == longs[0]
                            and scanned_pieces[g] == len(grps) - 1
                            and len(produced) < len(order)
                        ):
                            break
                        scan_piece(g, blks)
                        scanned_pieces[g] += 1

            for kb in order:
                produce(kb)
                produced.append(kb)
                try_scans()
            try_scans()
            assert shorts_done[0] or not NSH
            assert all(scanned_pieces[g] == len(pieces[g]) for g in longs)

            # final projection, ordered so the first long chunk (whose last
            # scan piece is the tail) comes last.
            fin_order = list(longs[1:]) + list(shorts) + [longs[0]]
            for i, g in enumerate(fin_order):
                nc.scalar.copy(mh_t[:, ds(g * BC, BC)], m_t[:, ds(g * BC, BC)])
                nc.tensor.matmul(
                    psy[:],
                    mh_t[:, ds(g * BC, BC)],
                    whot[g][:],
                    start=(i == 0),
                    stop=(i == NCH - 1),
                )

            y_t = consts.tile([BC, O], f32, tag="y", name="y_t")
            nc.vector.tensor_tensor(y_t[:], psy[:], bias_t[:], mybir.AluOpType.add)
            nc.sync.dma_start(Y, y_t[:])
    nc.compile()
    return nc


def _get_program(plan):
    key = (plan["kgs"], plan["longs"])
    if key not in _CACHE:
        _CACHE[key] = _build(key, plan)
    return _CACHE[key]


def _ensure_ntff_hook():
    """Provide antenv.axon_hooks (absent in this image) so trace=True works."""
    import sys
    import types

    if "antenv.axon_hooks" in sys.modules:
        return True
    try:
        import antenv

        mod = types.ModuleType("antenv.axon_hooks")
        mod._hook = None

        def set_axon_ntff_profile_hook(h):
            mod._hook = h

        def get_axon_ntff_profile_hook():
            return mod._hook

        mod.set_axon_ntff_profile_hook = set_axon_ntff_profile_hook
        mod.get_axon_ntff_profile_hook = get_axon_ntff_profile_hook
        sys.modules["antenv.axon_hooks"] = mod
        antenv.axon_hooks = mod

        from trn_agent_boot.trn_boot import _ntff_profile_via_ctypes

        hook = _ntff_profile_via_ctypes("/opt/axon/libaxon_pjrt.so")
        mod.set_axon_ntff_profile_hook(hook)
        return hook is not None
    except Exception:
        return False


def kernel(X, W_ih, hh, W_ho, b_ho):
    from concourse import bass_utils

    X = np.asarray(X, dtype=np.float32)
    W_ih = np.asarray(W_ih, dtype=np.float32)
    hh = np.asarray(hh, dtype=np.float32)
    W_ho = np.asarray(W_ho, dtype=np.float32)
    b_ho = np.asarray(b_ho, dtype=np.float32)

    plan = _make_plan(hh)
    perm = plan["perm"]
    nc = _get_program(plan)

    wiht = np.ascontiguousarray(W_ih[perm].T).astype(np.float16)  # [I, H]
    whot = np.ascontiguousarray(W_ho[:, perm].T).astype(np.float16)  # [H, O]
    bias = np.tile(b_ho[None, :], (BC, 1)).astype(np.float32)

    common = {"WIHT": wiht, "WHOT": whot, "BIAS": bias, "SCN": plan["SCN"]}
    in_maps = []
    for m in range(NCORES):
        im = dict(common)
        xm = X[:, m * BC : (m + 1) * BC, :]  # [S, BC, I]
        xt = xm.transpose(2, 1, 0).reshape(NI, 128, BC, NBLK, TB)
        xt = np.ascontiguousarray(xt.transpose(3, 0, 1, 2, 4)).reshape(
            NBLK, NI, 128, TB * BC
        )
        im["X"] = xt.astype(np.float16)
        in_maps.append(im)

    trace = bool(int(os.environ.get("DIAG_TRACE", "0")))
    if trace:
        trace = _ensure_ntff_hook()
    res = None
    for attempt in range(3):
        try:
            res = bass_utils.run_bass_kernel_spmd(
                nc,
                in_maps,
                core_ids=list(range(NCORES)),
                trace=trace,
                tmpdir=os.environ.get("DIAG_TRACE_DIR") or None,
            )
            break
        except Exception:
            if attempt == 2:
                raise
            trace = False  # retry without profiling
    if res.exec_time_ns is not None:
        kernel.last_exec_time_ns = res.exec_time_ns
        kernel.last_mean_exec_time_ns = res.mean_exec_time_ns
    Yfull = np.concatenate([r["Y"] for r in res.results], axis=0)
    return Yfull


kernel.last_exec_time_ns = None
kernel.last_mean_exec_time_ns = None
